# revision 4
# baseline (speedup 1.0000x reference)
"""GAT attention head (single head) on 8 Trainium2 NeuronCores.

Strategy (src-sharded CSR):
  - Nodes sharded into 8 contiguous ranges of 12500 (core c owns src range c).
  - Phase 1 (per core): h = x_shard @ W.T on PE; s1 = h@a1, s2 = h@a2 on DVE.
    Packed table rows [h(64)|1.0|junk|s2hi|s2lo] in bf16 written to an HBM
    shard, then AllGather -> full 100000-row table replicated per core.
  - Phase 2 (per core): nodes processed in degree-sorted 128-node tiles
    (CSR, padded to per-tile max degree, uniform schedule across cores).
    Per neighbor column j: indirect-DMA gather of 128 table rows (one row
    per partition), e = exp(leakyrelu(s1 + s2)) on ACT (s1 is a
    per-partition bias), rows scaled by e on DVE, accumulated into PSUM via
    identity matmuls on PE.  The embedded 1.0 column accumulates z = sum(e)
    for free; out rows = psum[:, :64] * (1/z).  Self loops use the locally
    kept rows (no gather).  Pad slots gather nothing (OOB-skip) and are
    zeroed via a host-provided edge mask.
  - No segment max: scores are bounded (~|s|<=10) so exp is safe in f32,
    and softmax is shift-invariant so the result matches the reference.
"""

import sys

if "/opt/trn_rl_repo" not in sys.path:
    sys.path.insert(0, "/opt/trn_rl_repo")

import numpy as np
import ml_dtypes

P = 128
C = 8
ROWW = 68          # table row: h[0:64], one@64, junk@65, s2hi@66, s2lo@67
SCALE_COLS = 66    # columns 0..65 are scaled by e and fed to the matmul
NEG_SLOPE = 0.2


def _preprocess(edge_index, N):
    """Host-side index preprocessing. Returns per-core arrays + schedule."""
    NS = N // C
    T = (NS + P - 1) // P
    src = np.asarray(edge_index[0], dtype=np.int64)
    dst = np.asarray(edge_index[1], dtype=np.int64)
    E = src.shape[0]

    degN = np.bincount(src, minlength=N)

    perm_slots = np.empty(N, dtype=np.int64)   # orig node -> global table row
    orders = []                                # per core: slot -> local node
    tilemax = np.zeros((C, T), dtype=np.int64)
    for c in range(C):
        d = degN[c * NS:(c + 1) * NS]
        order = np.argsort(-d, kind="stable")
        orders.append(order)
        inv = np.empty(NS, dtype=np.int64)
        inv[order] = np.arange(NS)
        perm_slots[c * NS:(c + 1) * NS] = c * NS + inv
        ds = d[order]
        for t in range(T):
            seg = ds[t * P:(t + 1) * P]
            tilemax[c, t] = int(seg.max()) if seg.size else 0

    degpad = tilemax.max(axis=0).astype(np.int64)   # uniform schedule
    base = np.concatenate([[0], np.cumsum(degpad)])
    SUMDEG = int(base[-1])

    offs = np.full((C, P, SUMDEG), N, dtype=np.int32)  # N = OOB sentinel
    msk = np.zeros((C, P, SUMDEG), dtype=np.float32)

    gslot = perm_slots[src]
    eorder = np.argsort(gslot, kind="stable")
    gs = gslot[eorder]
    ed = dst[eorder]
    cnt = np.bincount(gs, minlength=N)
    starts = np.concatenate([[0], np.cumsum(cnt)])
    j_in = np.arange(E, dtype=np.int64) - starts[gs]

    c_of = gs // NS
    ls = gs % NS
    tt = ls // P
    pp = ls % P
    col = base[tt] + j_in
    offs[c_of, pp, col] = perm_slots[ed].astype(np.int32)
    msk[c_of, pp, col] = 1.0

    return dict(NS=NS, T=T, degpad=degpad, base=base, SUMDEG=SUMDEG,
                offs=offs, msk=msk.astype(ml_dtypes.bfloat16), orders=orders)


def _build_program(N, IN, OUT, NS, T, degpad, base, SUMDEG):
    from concourse import bass, bacc, mybir, tile

    f32 = mybir.dt.float32
    bf16 = mybir.dt.bfloat16
    i32 = mybir.dt.int32
    AT = mybir.ActivationFunctionType
    OP = mybir.AluOpType
    X = mybir.AxisListType.X

    NPAD = T * P
    nc = bacc.Bacc("TRN2", target_bir_lowering=False, debug=False)

    xT = nc.dram_tensor("xT", [IN, NPAD], f32, kind="ExternalInput")
    wT = nc.dram_tensor("wT", [IN, OUT], f32, kind="ExternalInput")
    a1b = nc.dram_tensor("a1b", [P, OUT], f32, kind="ExternalInput")
    a2b = nc.dram_tensor("a2b", [P, OUT], f32, kind="ExternalInput")
    ident = nc.dram_tensor("ident", [P, P], bf16, kind="ExternalInput")
    offs = nc.dram_tensor("offs", [P, SUMDEG], i32, kind="ExternalInput")
    msk = nc.dram_tensor("msk", [P, SUMDEG], bf16, kind="ExternalInput")
    outd = nc.dram_tensor("out", [NS, OUT], f32, kind="ExternalOutput")

    shard = nc.dram_tensor("shard", [NS, ROWW], bf16)
    full = nc.dram_tensor("full", [C * NS, ROWW], bf16, addr_space="Shared")

    DMAX = int(max(degpad.max(), 1))

    with tile.TileContext(nc) as tc:
        with (
            tc.tile_pool(name="const", bufs=1) as cp,
            tc.tile_pool(name="xp", bufs=3) as xp,
            tc.tile_pool(name="tmp", bufs=3) as tp,
            tc.tile_pool(name="gp", bufs=3) as gp,
            tc.tile_pool(name="hp", bufs=4) as hpp,
            tc.tile_pool(name="ep", bufs=3) as ep,
            tc.tile_pool(name="ob", bufs=3) as obp,
            tc.tile_pool(name="ps", bufs=4, space="PSUM") as psp,
            tc.tile_pool(name="ps1", bufs=2, space="PSUM") as psp1,
        ):
            ident_sb = cp.tile([P, P], bf16)
            nc.sync.dma_start(ident_sb[:], ident[:])
            wt0 = cp.tile([P, OUT], f32)
            wt1 = cp.tile([P, OUT], f32)
            nc.sync.dma_start(wt0[:], wT[0:P, :])
            nc.sync.dma_start(wt1[:], wT[P:2 * P, :])
            a1_sb = cp.tile([P, OUT], f32)
            a2_sb = cp.tile([P, OUT], f32)
            nc.sync.dma_start(a1_sb[:], a1b[:])
            nc.sync.dma_start(a2_sb[:], a2b[:])
            offs_sb = cp.tile([P, SUMDEG], i32)
            nc.sync.dma_start(offs_sb[:], offs[:])
            msk_sb = cp.tile([P, SUMDEG], bf16)
            nc.sync.dma_start(msk_sb[:], msk[:])

            own_sb = cp.tile([P, T * ROWW], bf16)
            s1_sb = cp.tile([P, T], f32)
            s2_sb = cp.tile([P, T], f32)
            es_sb = cp.tile([P, T], f32)

            # ---------------- phase 1: h, s1, s2, table shard ----------------
            for t in range(T):
                rows = min(P, NS - t * P)
                xa = xp.tile([P, P], f32, tag="xa")
                xb = xp.tile([P, P], f32, tag="xb")
                nc.sync.dma_start(xa[:], xT[0:P, t * P:(t + 1) * P])
                nc.sync.dma_start(xb[:], xT[P:2 * P, t * P:(t + 1) * P])
                ph = psp1.tile([P, OUT], f32)
                nc.tensor.matmul(ph[:], xa[:], wt0[:], start=True, stop=False)
                nc.tensor.matmul(ph[:], xb[:], wt1[:], start=False, stop=True)

                tmp = tp.tile([P, OUT], f32, tag="tmp")
                nc.vector.tensor_tensor(tmp[:], ph[:], a1_sb[:], OP.mult)
                nc.vector.tensor_reduce(s1_sb[:, t:t + 1], tmp[:], X, OP.add)
                nc.vector.tensor_tensor(tmp[:], ph[:], a2_sb[:], OP.mult)
                nc.vector.tensor_reduce(s2_sb[:, t:t + 1], tmp[:], X, OP.add)

                orow = own_sb[:, t * ROWW:(t + 1) * ROWW]
                nc.vector.tensor_copy(orow[:, 0:OUT], ph[:])
                nc.vector.memset(orow[:, OUT:OUT + 1], 1.0)
                nc.vector.memset(orow[:, OUT + 1:OUT + 2], 0.0)
                nc.vector.tensor_copy(orow[:, 66:67], s2_sb[:, t:t + 1])
                lo = tp.tile([P, 1], f32, tag="lo")
                nc.vector.tensor_tensor(lo[:], s2_sb[:, t:t + 1], orow[:, 66:67],
                                        OP.subtract)
                nc.vector.tensor_copy(orow[:, 67:68], lo[:])
                nc.sync.dma_start(shard[t * P:t * P + rows, :], orow[:rows, :])

            # ---------------- all-gather the table ----------------
            nc.gpsimd.collective_compute(
                "AllGather", mybir.AluOpType.bypass,
                replica_groups=[list(range(C))],
                ins=[shard[:].opt()], outs=[full[:].opt()])

            # self-loop scores for all tiles at once
            ss = cp.tile([P, T], f32)
            ss2 = cp.tile([P, T], f32)
            nc.vector.tensor_tensor(ss[:], s1_sb[:], s2_sb[:], OP.add)
            nc.vector.tensor_scalar(ss2[:], ss[:], NEG_SLOPE, None, OP.mult)
            nc.vector.tensor_tensor(ss[:], ss[:], ss2[:], OP.max)
            nc.scalar.activation(es_sb[:], ss[:], AT.Exp)

            # ---------------- phase 2: gather, softmax, aggregate ----------------
            for t in range(T):
                rows = min(P, NS - t * P)
                D = int(degpad[t])
                b0 = int(base[t])
                g = gp.tile([P, max(D, 1) * ROWW], bf16, tag="g")
                if t < 3:
                    # first use of each rotating slot: clear stale/NaN bits
                    nc.vector.memset(g[:], 0.0)
                for j in range(D):
                    nc.gpsimd.indirect_dma_start(
                        out=g[:, j * ROWW:(j + 1) * ROWW],
                        out_offset=None,
                        in_=full[:],
                        in_offset=bass.IndirectOffsetOnAxis(
                            ap=offs_sb[:, b0 + j:b0 + j + 1], axis=0),
                        bounds_check=C * NS - 1,
                        oob_is_err=False)

                pt = psp.tile([P, SCALE_COLS], f32)
                if D > 0:
                    ev = ep.tile([P, D], f32, tag="ev")
                    ev2 = ep.tile([P, D], f32, tag="ev2")
                    hi_ap = g[:].rearrange("p (d r) -> p d r", r=ROWW)
                    nc.vector.tensor_tensor(
                        ev[:], hi_ap[:, :, 66], hi_ap[:, :, 67], OP.add)
                    nc.vector.tensor_scalar(ev[:], ev[:], s1_sb[:, t:t + 1],
                                            None, OP.add)
                    nc.vector.tensor_scalar(ev2[:], ev[:], NEG_SLOPE, None,
                                            OP.mult)
                    nc.vector.tensor_tensor(ev[:], ev[:], ev2[:], OP.max)
                    nc.scalar.activation(ev[:], ev[:], AT.Exp)
                    nc.vector.tensor_tensor(ev[:], ev[:],
                                            msk_sb[:, b0:b0 + D], OP.mult)
                    for j in range(D):
                        hp = hpp.tile([P, SCALE_COLS], bf16, tag="hp")
                        nc.vector.tensor_scalar(
                            hp[:], g[:, j * ROWW:j * ROWW + SCALE_COLS],
                            ev[:, j:j + 1], None, OP.mult)
                        nc.tensor.matmul(pt[:], ident_sb[:], hp[:],
                                         start=(j == 0), stop=False)

                hs = hpp.tile([P, SCALE_COLS], bf16, tag="hp")
                nc.vector.tensor_scalar(
                    hs[:], own_sb[:, t * ROWW:t * ROWW + SCALE_COLS],
                    es_sb[:, t:t + 1], None, OP.mult)
                nc.tensor.matmul(pt[:], ident_sb[:], hs[:],
                                 start=(D == 0), stop=True)

                rz = ep.tile([P, 1], f32, tag="rz")
                nc.vector.reciprocal(rz[:], pt[:, OUT:OUT + 1])
                ob = obp.tile([P, OUT], f32, tag="ob")
                nc.vector.tensor_scalar(ob[:], pt[:, 0:OUT], rz[:], None,
                                        OP.mult)
                nc.sync.dma_start(outd[t * P:t * P + rows, :], ob[:rows, :])

    nc.compile()
    return nc


def kernel(x, edge_index, W, a1, a2):
    from concourse.bass_utils import run_bass_kernel_spmd

    x = np.asarray(x)
    edge_index = np.asarray(edge_index)
    W = np.asarray(W)
    a1 = np.asarray(a1)
    a2 = np.asarray(a2)

    N, IN = x.shape
    OUT = W.shape[0]
    pre = _preprocess(edge_index, N)
    NS, T = pre["NS"], pre["T"]
    NPAD = T * P

    nc = _build_program(N, IN, OUT, NS, T, pre["degpad"], pre["base"],
                        pre["SUMDEG"])

    wTn = np.ascontiguousarray(W.T.astype(np.float32))
    a1b = np.ascontiguousarray(np.tile(a1.astype(np.float32), (P, 1)))
    a2b = np.ascontiguousarray(np.tile(a2.astype(np.float32), (P, 1)))
    identn = np.eye(P, dtype=np.float32).astype(ml_dtypes.bfloat16)

    in_maps = []
    for c in range(C):
        orig = c * NS + pre["orders"][c]          # slot -> orig node id
        xs = np.zeros((NPAD, IN), dtype=np.float32)
        xs[:NS] = x[orig]
        in_maps.append({
            "xT": np.ascontiguousarray(xs.T),
            "wT": wTn,
            "a1b": a1b,
            "a2b": a2b,
            "ident": identn,
            "offs": np.ascontiguousarray(pre["offs"][c]),
            "msk": np.ascontiguousarray(pre["msk"][c]),
        })

    res = run_bass_kernel_spmd(nc, in_maps, list(range(C)))

    out = np.empty((N, OUT), dtype=np.float32)
    for c in range(C):
        orig = c * NS + pre["orders"][c]
        out[orig] = res.results[c]["out"]
    return out


# revision 5
# speedup vs baseline: 1.0333x; 1.0333x over previous
"""GAT attention head (single head) on 8 Trainium2 NeuronCores.

Strategy (src-sharded CSR):
  - Nodes sharded into 8 contiguous ranges of 12500 (core c owns src range c).
  - Phase 1 (per core): h = x_shard @ W.T on PE; s1 = h@a1, s2 = h@a2 on DVE.
    Packed table rows [h(64)|1.0|junk|s2hi|s2lo] in bf16 written to an HBM
    shard, then AllGather -> full 100000-row table replicated per core.
  - Phase 2 (per core): nodes processed in degree-sorted 128-node tiles
    (CSR, padded to per-tile max degree, uniform schedule across cores).
    Per neighbor column j: indirect-DMA gather of 128 table rows (one row
    per partition), e = exp(leakyrelu(s1 + s2)) on ACT (s1 is a
    per-partition bias), rows scaled by e on DVE, accumulated into PSUM via
    identity matmuls on PE.  The embedded 1.0 column accumulates z = sum(e)
    for free; out rows = psum[:, :64] * (1/z).  Self loops use the locally
    kept rows (no gather).  Pad slots gather nothing (OOB-skip) and are
    zeroed via a host-provided edge mask.
  - No segment max: scores are bounded (~|s|<=10) so exp is safe in f32,
    and softmax is shift-invariant so the result matches the reference.
"""

import sys

if "/opt/trn_rl_repo" not in sys.path:
    sys.path.insert(0, "/opt/trn_rl_repo")

import numpy as np
import ml_dtypes

P = 128
C = 8
ROWW = 68          # table row: h[0:64], one@64, junk@65, s2hi@66, s2lo@67
SCALE_COLS = 66    # columns 0..65 are scaled by e and fed to the matmul
NEG_SLOPE = 0.2


def _preprocess(edge_index, N):
    """Host-side index preprocessing. Returns per-core arrays + schedule."""
    NS = N // C
    T = (NS + P - 1) // P
    src = np.asarray(edge_index[0], dtype=np.int64)
    dst = np.asarray(edge_index[1], dtype=np.int64)
    E = src.shape[0]

    degN = np.bincount(src, minlength=N)

    perm_slots = np.empty(N, dtype=np.int64)   # orig node -> global table row
    orders = []                                # per core: slot -> local node
    tilemax = np.zeros((C, T), dtype=np.int64)
    for c in range(C):
        d = degN[c * NS:(c + 1) * NS]
        order = np.argsort(-d, kind="stable")
        orders.append(order)
        inv = np.empty(NS, dtype=np.int64)
        inv[order] = np.arange(NS)
        perm_slots[c * NS:(c + 1) * NS] = c * NS + inv
        ds = d[order]
        for t in range(T):
            seg = ds[t * P:(t + 1) * P]
            tilemax[c, t] = int(seg.max()) if seg.size else 0

    degpad = tilemax.max(axis=0).astype(np.int64)   # uniform schedule
    base = np.concatenate([[0], np.cumsum(degpad)])
    SUMDEG = int(base[-1])

    offs = np.zeros((C, P, SUMDEG), dtype=np.int32)  # pads gather row 0; mask kills e
    msk = np.zeros((C, P, SUMDEG), dtype=np.float32)

    gslot = perm_slots[src]
    eorder = np.argsort(gslot, kind="stable")
    gs = gslot[eorder]
    ed = dst[eorder]
    cnt = np.bincount(gs, minlength=N)
    starts = np.concatenate([[0], np.cumsum(cnt)])
    j_in = np.arange(E, dtype=np.int64) - starts[gs]

    c_of = gs // NS
    ls = gs % NS
    tt = ls // P
    pp = ls % P
    col = base[tt] + j_in
    offs[c_of, pp, col] = perm_slots[ed].astype(np.int32)
    msk[c_of, pp, col] = 1.0

    return dict(NS=NS, T=T, degpad=degpad, base=base, SUMDEG=SUMDEG,
                offs=offs, msk=msk.astype(ml_dtypes.bfloat16), orders=orders)


def _build_program(N, IN, OUT, NS, T, degpad, base, SUMDEG):
    from concourse import bass, bacc, mybir, tile

    f32 = mybir.dt.float32
    bf16 = mybir.dt.bfloat16
    i32 = mybir.dt.int32
    AT = mybir.ActivationFunctionType
    OP = mybir.AluOpType
    X = mybir.AxisListType.X

    NPAD = T * P
    nc = bacc.Bacc("TRN2", target_bir_lowering=False, debug=False)

    xT = nc.dram_tensor("xT", [IN, NPAD], f32, kind="ExternalInput")
    wT = nc.dram_tensor("wT", [IN, OUT + 2], f32, kind="ExternalInput")
    ident = nc.dram_tensor("ident", [P, P], bf16, kind="ExternalInput")
    offs = nc.dram_tensor("offs", [P, SUMDEG], i32, kind="ExternalInput")
    msk = nc.dram_tensor("msk", [P, SUMDEG], bf16, kind="ExternalInput")
    outd = nc.dram_tensor("out", [NS, OUT], f32, kind="ExternalOutput")

    shard = nc.dram_tensor("shard", [NS, ROWW], bf16)
    full = nc.dram_tensor("full", [C * NS, ROWW], bf16, addr_space="Shared")

    DMAX = int(max(degpad.max(), 1))

    with tile.TileContext(nc) as tc:
        with (
            tc.tile_pool(name="const", bufs=1) as cp,
            tc.tile_pool(name="xp", bufs=3) as xp,
            tc.tile_pool(name="tmp", bufs=3) as tp,
            tc.tile_pool(name="gp", bufs=3) as gp,
            tc.tile_pool(name="hp", bufs=4) as hpp,
            tc.tile_pool(name="ep", bufs=3) as ep,
            tc.tile_pool(name="ob", bufs=3) as obp,
            tc.tile_pool(name="ps", bufs=4, space="PSUM") as psp,
            tc.tile_pool(name="ps1", bufs=4, space="PSUM") as psp1,
        ):
            ident_sb = cp.tile([P, P], bf16)
            nc.sync.dma_start(ident_sb[:], ident[:])
            wt0 = cp.tile([P, OUT + 2], f32)
            wt1 = cp.tile([P, OUT + 2], f32)
            nc.sync.dma_start(wt0[:], wT[0:P, :])
            nc.sync.dma_start(wt1[:], wT[P:2 * P, :])
            offs_sb = cp.tile([P, SUMDEG], i32)
            nc.sync.dma_start(offs_sb[:], offs[:])
            msk_sb = cp.tile([P, SUMDEG], bf16)
            nc.sync.dma_start(msk_sb[:], msk[:])

            own_sb = cp.tile([P, T * ROWW], bf16)
            s1_sb = cp.tile([P, T], f32)
            s2_sb = cp.tile([P, T], f32)
            es_sb = cp.tile([P, T], f32)

            # ---------------- phase 1: h, s1, s2, table shard ----------------
            for t in range(T):
                rows = min(P, NS - t * P)
                xa = xp.tile([P, P], f32, tag="xa")
                xb = xp.tile([P, P], f32, tag="xb")
                nc.sync.dma_start(xa[:], xT[0:P, t * P:(t + 1) * P])
                nc.sync.dma_start(xb[:], xT[P:2 * P, t * P:(t + 1) * P])
                ph = psp1.tile([P, OUT + 2], f32)
                nc.tensor.matmul(ph[:], xa[:], wt0[:], start=True, stop=False)
                nc.tensor.matmul(ph[:], xb[:], wt1[:], start=False, stop=True)

                nc.vector.tensor_copy(s1_sb[:, t:t + 1], ph[:, OUT:OUT + 1])
                nc.vector.tensor_copy(s2_sb[:, t:t + 1], ph[:, OUT + 1:OUT + 2])

                orow = own_sb[:, t * ROWW:(t + 1) * ROWW]
                nc.scalar.copy(orow[:, 0:OUT], ph[:, 0:OUT])
                nc.vector.memset(orow[:, OUT:OUT + 1], 1.0)
                nc.vector.memset(orow[:, OUT + 1:OUT + 2], 0.0)
                nc.vector.tensor_copy(orow[:, 66:67], s2_sb[:, t:t + 1])
                lo = tp.tile([P, 1], f32, tag="lo")
                nc.vector.tensor_tensor(lo[:], s2_sb[:, t:t + 1], orow[:, 66:67],
                                        OP.subtract)
                nc.vector.tensor_copy(orow[:, 67:68], lo[:])
                nc.sync.dma_start(shard[t * P:t * P + rows, :], orow[:rows, :])

            # ---------------- all-gather the table ----------------
            nc.gpsimd.collective_compute(
                "AllGather", mybir.AluOpType.bypass,
                replica_groups=[list(range(C))],
                ins=[shard[:].opt()], outs=[full[:].opt()])

            # self-loop scores for all tiles at once
            ss = cp.tile([P, T], f32)
            ss2 = cp.tile([P, T], f32)
            nc.vector.tensor_tensor(ss[:], s1_sb[:], s2_sb[:], OP.add)
            nc.vector.tensor_scalar(ss2[:], ss[:], NEG_SLOPE, None, OP.mult)
            nc.vector.tensor_tensor(ss[:], ss[:], ss2[:], OP.max)
            nc.scalar.activation(es_sb[:], ss[:], AT.Exp)

            # ---------------- phase 2: gather, softmax, aggregate ----------------
            for t in range(T):
                rows = min(P, NS - t * P)
                D = int(degpad[t])
                b0 = int(base[t])
                g = gp.tile([P, max(D, 1) * ROWW], bf16, tag="g")
                for j in range(D):
                    nc.gpsimd.indirect_dma_start(
                        out=g[:, j * ROWW:(j + 1) * ROWW],
                        out_offset=None,
                        in_=full[:],
                        in_offset=bass.IndirectOffsetOnAxis(
                            ap=offs_sb[:, b0 + j:b0 + j + 1], axis=0))

                pt = psp.tile([P, SCALE_COLS], f32)
                if D > 0:
                    ev = ep.tile([P, D], f32, tag="ev")
                    ev2 = ep.tile([P, D], f32, tag="ev2")
                    hi_ap = g[:].rearrange("p (d r) -> p d r", r=ROWW)
                    nc.vector.tensor_tensor(
                        ev[:], hi_ap[:, :, 66], hi_ap[:, :, 67], OP.add)
                    nc.vector.tensor_scalar(ev[:], ev[:], s1_sb[:, t:t + 1],
                                            None, OP.add)
                    nc.vector.tensor_scalar(ev2[:], ev[:], NEG_SLOPE, None,
                                            OP.mult)
                    nc.vector.tensor_tensor(ev[:], ev[:], ev2[:], OP.max)
                    nc.scalar.activation(ev[:], ev[:], AT.Exp)
                    nc.vector.tensor_tensor(ev[:], ev[:],
                                            msk_sb[:, b0:b0 + D], OP.mult)
                    for j in range(D):
                        hp = hpp.tile([P, SCALE_COLS], bf16, tag="hp")
                        nc.vector.tensor_scalar(
                            hp[:], g[:, j * ROWW:j * ROWW + SCALE_COLS],
                            ev[:, j:j + 1], None, OP.mult)
                        nc.tensor.matmul(pt[:], ident_sb[:], hp[:],
                                         start=(j == 0), stop=False)

                hs = hpp.tile([P, SCALE_COLS], bf16, tag="hp")
                nc.vector.tensor_scalar(
                    hs[:], own_sb[:, t * ROWW:t * ROWW + SCALE_COLS],
                    es_sb[:, t:t + 1], None, OP.mult)
                nc.tensor.matmul(pt[:], ident_sb[:], hs[:],
                                 start=(D == 0), stop=True)

                rz = ep.tile([P, 1], f32, tag="rz")
                nc.vector.reciprocal(rz[:], pt[:, OUT:OUT + 1])
                ob = obp.tile([P, OUT], f32, tag="ob")
                nc.vector.tensor_scalar(ob[:], pt[:, 0:OUT], rz[:], None,
                                        OP.mult)
                nc.sync.dma_start(outd[t * P:t * P + rows, :], ob[:rows, :])

    nc.compile()
    return nc


def kernel(x, edge_index, W, a1, a2):
    from concourse.bass_utils import run_bass_kernel_spmd

    x = np.asarray(x)
    edge_index = np.asarray(edge_index)
    W = np.asarray(W)
    a1 = np.asarray(a1)
    a2 = np.asarray(a2)

    N, IN = x.shape
    OUT = W.shape[0]
    pre = _preprocess(edge_index, N)
    NS, T = pre["NS"], pre["T"]
    NPAD = T * P

    nc = _build_program(N, IN, OUT, NS, T, pre["degpad"], pre["base"],
                        pre["SUMDEG"])

    Wt = W.T.astype(np.float32)
    wTn = np.ascontiguousarray(np.concatenate(
        [Wt, (Wt @ a1.astype(np.float32))[:, None],
         (Wt @ a2.astype(np.float32))[:, None]], axis=1))
    identn = np.eye(P, dtype=np.float32).astype(ml_dtypes.bfloat16)

    in_maps = []
    for c in range(C):
        orig = c * NS + pre["orders"][c]          # slot -> orig node id
        xs = np.zeros((NPAD, IN), dtype=np.float32)
        xs[:NS] = x[orig]
        in_maps.append({
            "xT": np.ascontiguousarray(xs.T),
            "wT": wTn,
            "ident": identn,
            "offs": np.ascontiguousarray(pre["offs"][c]),
            "msk": np.ascontiguousarray(pre["msk"][c]),
        })

    res = run_bass_kernel_spmd(nc, in_maps, list(range(C)))

    out = np.empty((N, OUT), dtype=np.float32)
    for c in range(C):
        orig = c * NS + pre["orders"][c]
        out[orig] = res.results[c]["out"]
    return out


# revision 6
# speedup vs baseline: 1.0343x; 1.0010x over previous
"""GAT attention head (single head) on 8 Trainium2 NeuronCores.

Strategy (src-sharded CSR):
  - Nodes sharded into 8 contiguous ranges of 12500 (core c owns src range c).
  - Phase 1 (per core): h = x_shard @ W.T on PE; s1 = h@a1, s2 = h@a2 on DVE.
    Packed table rows [h(64)|1.0|junk|s2hi|s2lo] in bf16 written to an HBM
    shard, then AllGather -> full 100000-row table replicated per core.
  - Phase 2 (per core): nodes processed in degree-sorted 128-node tiles
    (CSR, padded to per-tile max degree, uniform schedule across cores).
    Per neighbor column j: indirect-DMA gather of 128 table rows (one row
    per partition), e = exp(leakyrelu(s1 + s2)) on ACT (s1 is a
    per-partition bias), rows scaled by e on DVE, accumulated into PSUM via
    identity matmuls on PE.  The embedded 1.0 column accumulates z = sum(e)
    for free; out rows = psum[:, :64] * (1/z).  Self loops use the locally
    kept rows (no gather).  Pad slots gather nothing (OOB-skip) and are
    zeroed via a host-provided edge mask.
  - No segment max: scores are bounded (~|s|<=10) so exp is safe in f32,
    and softmax is shift-invariant so the result matches the reference.
"""

import sys

if "/opt/trn_rl_repo" not in sys.path:
    sys.path.insert(0, "/opt/trn_rl_repo")

import numpy as np
import ml_dtypes

P = 128
C = 8
ROWW = 68          # table row: h[0:64], one@64, junk@65, s2hi@66, s2lo@67
SCALE_COLS = 66    # columns 0..65 are scaled by e and fed to the matmul
NEG_SLOPE = 0.2


def _preprocess(edge_index, N):
    """Host-side index preprocessing. Returns per-core arrays + schedule."""
    NS = N // C
    T = (NS + P - 1) // P
    src = np.asarray(edge_index[0], dtype=np.int64)
    dst = np.asarray(edge_index[1], dtype=np.int64)
    E = src.shape[0]

    degN = np.bincount(src, minlength=N)

    perm_slots = np.empty(N, dtype=np.int64)   # orig node -> global table row
    orders = []                                # per core: slot -> local node
    tilemax = np.zeros((C, T), dtype=np.int64)
    for c in range(C):
        d = degN[c * NS:(c + 1) * NS]
        order = np.argsort(-d, kind="stable")
        orders.append(order)
        inv = np.empty(NS, dtype=np.int64)
        inv[order] = np.arange(NS)
        perm_slots[c * NS:(c + 1) * NS] = c * NS + inv
        ds = d[order]
        for t in range(T):
            seg = ds[t * P:(t + 1) * P]
            tilemax[c, t] = int(seg.max()) if seg.size else 0

    degpad = tilemax.max(axis=0).astype(np.int64)   # uniform schedule
    base = np.concatenate([[0], np.cumsum(degpad)])
    SUMDEG = int(base[-1])

    offs = np.zeros((C, P, SUMDEG), dtype=np.int32)  # pads gather row 0; mask kills e
    msk = np.zeros((C, P, SUMDEG), dtype=np.float32)

    gslot = perm_slots[src]
    eorder = np.argsort(gslot, kind="stable")
    gs = gslot[eorder]
    ed = dst[eorder]
    cnt = np.bincount(gs, minlength=N)
    starts = np.concatenate([[0], np.cumsum(cnt)])
    j_in = np.arange(E, dtype=np.int64) - starts[gs]

    c_of = gs // NS
    ls = gs % NS
    tt = ls // P
    pp = ls % P
    col = base[tt] + j_in
    offs[c_of, pp, col] = perm_slots[ed].astype(np.int32)
    msk[c_of, pp, col] = 1.0

    return dict(NS=NS, T=T, degpad=degpad, base=base, SUMDEG=SUMDEG,
                offs=offs, msk=msk.astype(ml_dtypes.bfloat16), orders=orders)


def _build_program(N, IN, OUT, NS, T, degpad, base, SUMDEG):
    from concourse import bass, bacc, mybir, tile

    f32 = mybir.dt.float32
    bf16 = mybir.dt.bfloat16
    i32 = mybir.dt.int32
    AT = mybir.ActivationFunctionType
    OP = mybir.AluOpType
    X = mybir.AxisListType.X

    NPAD = T * P
    nc = bacc.Bacc("TRN2", target_bir_lowering=False, debug=False)

    xT = nc.dram_tensor("xT", [IN, NPAD], bf16, kind="ExternalInput")
    wT = nc.dram_tensor("wT", [IN, OUT + 2], bf16, kind="ExternalInput")
    ident = nc.dram_tensor("ident", [P, P], bf16, kind="ExternalInput")
    offs = nc.dram_tensor("offs", [P, SUMDEG], i32, kind="ExternalInput")
    msk = nc.dram_tensor("msk", [P, SUMDEG], bf16, kind="ExternalInput")
    outd = nc.dram_tensor("out", [NS, OUT], f32, kind="ExternalOutput")

    shard = nc.dram_tensor("shard", [NS, ROWW], bf16)
    full = nc.dram_tensor("full", [C * NS, ROWW], bf16, addr_space="Shared")

    DMAX = int(max(degpad.max(), 1))

    with tile.TileContext(nc) as tc:
        with (
            tc.tile_pool(name="const", bufs=1) as cp,
            tc.tile_pool(name="xp", bufs=3) as xp,
            tc.tile_pool(name="tmp", bufs=3) as tp,
            tc.tile_pool(name="gp", bufs=3) as gp,
            tc.tile_pool(name="hp", bufs=4) as hpp,
            tc.tile_pool(name="ep", bufs=3) as ep,
            tc.tile_pool(name="ob", bufs=3) as obp,
            tc.tile_pool(name="ps", bufs=4, space="PSUM") as psp,
            tc.tile_pool(name="ps1", bufs=4, space="PSUM") as psp1,
        ):
            ident_sb = cp.tile([P, P], bf16)
            nc.sync.dma_start(ident_sb[:], ident[:])
            wt0 = cp.tile([P, OUT + 2], bf16)
            wt1 = cp.tile([P, OUT + 2], bf16)
            nc.sync.dma_start(wt0[:], wT[0:P, :])
            nc.sync.dma_start(wt1[:], wT[P:2 * P, :])
            offs_sb = cp.tile([P, SUMDEG], i32)
            nc.sync.dma_start(offs_sb[:], offs[:])
            msk_sb = cp.tile([P, SUMDEG], bf16)
            nc.sync.dma_start(msk_sb[:], msk[:])

            own_sb = cp.tile([P, T * ROWW], bf16)
            s1_sb = cp.tile([P, T], f32)
            s2_sb = cp.tile([P, T], f32)
            es_sb = cp.tile([P, T], f32)

            # ---------------- phase 1: h, s1, s2, table shard ----------------
            for t in range(T):
                rows = min(P, NS - t * P)
                xa = xp.tile([P, P], bf16, tag="xa")
                xb = xp.tile([P, P], bf16, tag="xb")
                nc.sync.dma_start(xa[:], xT[0:P, t * P:(t + 1) * P])
                nc.sync.dma_start(xb[:], xT[P:2 * P, t * P:(t + 1) * P])
                ph = psp1.tile([P, OUT + 2], f32)
                nc.tensor.matmul(ph[:], xa[:], wt0[:], start=True, stop=False)
                nc.tensor.matmul(ph[:], xb[:], wt1[:], start=False, stop=True)

                nc.vector.tensor_copy(s1_sb[:, t:t + 1], ph[:, OUT:OUT + 1])
                nc.vector.tensor_copy(s2_sb[:, t:t + 1], ph[:, OUT + 1:OUT + 2])

                orow = own_sb[:, t * ROWW:(t + 1) * ROWW]
                nc.scalar.copy(orow[:, 0:OUT], ph[:, 0:OUT])
                nc.vector.memset(orow[:, OUT:OUT + 1], 1.0)
                nc.vector.memset(orow[:, OUT + 1:OUT + 2], 0.0)
                nc.vector.tensor_copy(orow[:, 66:67], s2_sb[:, t:t + 1])
                lo = tp.tile([P, 1], f32, tag="lo")
                nc.vector.tensor_tensor(lo[:], s2_sb[:, t:t + 1], orow[:, 66:67],
                                        OP.subtract)
                nc.vector.tensor_copy(orow[:, 67:68], lo[:])
                nc.sync.dma_start(shard[t * P:t * P + rows, :], orow[:rows, :])

            # ---------------- all-gather the table ----------------
            nc.gpsimd.collective_compute(
                "AllGather", mybir.AluOpType.bypass,
                replica_groups=[list(range(C))],
                ins=[shard[:].opt()], outs=[full[:].opt()])

            # self-loop scores for all tiles at once
            ss = cp.tile([P, T], f32)
            ss2 = cp.tile([P, T], f32)
            nc.vector.tensor_tensor(ss[:], s1_sb[:], s2_sb[:], OP.add)
            nc.vector.tensor_scalar(ss2[:], ss[:], NEG_SLOPE, None, OP.mult)
            nc.vector.tensor_tensor(ss[:], ss[:], ss2[:], OP.max)
            nc.scalar.activation(es_sb[:], ss[:], AT.Exp)

            # ---------------- phase 2: gather, softmax, aggregate ----------------
            for t in range(T):
                rows = min(P, NS - t * P)
                D = int(degpad[t])
                b0 = int(base[t])
                g = gp.tile([P, max(D, 1) * ROWW], bf16, tag="g")
                for j in range(D):
                    nc.gpsimd.indirect_dma_start(
                        out=g[:, j * ROWW:(j + 1) * ROWW],
                        out_offset=None,
                        in_=full[:],
                        in_offset=bass.IndirectOffsetOnAxis(
                            ap=offs_sb[:, b0 + j:b0 + j + 1], axis=0))

                pt = psp.tile([P, SCALE_COLS], f32)
                if D > 0:
                    ev = ep.tile([P, D], f32, tag="ev")
                    ev2 = ep.tile([P, D], f32, tag="ev2")
                    hi_ap = g[:].rearrange("p (d r) -> p d r", r=ROWW)
                    nc.vector.tensor_tensor(
                        ev[:], hi_ap[:, :, 66], hi_ap[:, :, 67], OP.add)
                    nc.vector.tensor_scalar(ev[:], ev[:], s1_sb[:, t:t + 1],
                                            None, OP.add)
                    nc.vector.tensor_scalar(ev2[:], ev[:], NEG_SLOPE, None,
                                            OP.mult)
                    nc.vector.tensor_tensor(ev[:], ev[:], ev2[:], OP.max)
                    nc.scalar.activation(ev[:], ev[:], AT.Exp)
                    nc.vector.tensor_tensor(ev[:], ev[:],
                                            msk_sb[:, b0:b0 + D], OP.mult)
                    for j in range(D):
                        hp = hpp.tile([P, SCALE_COLS], bf16, tag="hp")
                        nc.vector.tensor_scalar(
                            hp[:], g[:, j * ROWW:j * ROWW + SCALE_COLS],
                            ev[:, j:j + 1], None, OP.mult)
                        nc.tensor.matmul(pt[:], ident_sb[:], hp[:],
                                         start=(j == 0), stop=False)

                hs = hpp.tile([P, SCALE_COLS], bf16, tag="hp")
                nc.vector.tensor_scalar(
                    hs[:], own_sb[:, t * ROWW:t * ROWW + SCALE_COLS],
                    es_sb[:, t:t + 1], None, OP.mult)
                nc.tensor.matmul(pt[:], ident_sb[:], hs[:],
                                 start=(D == 0), stop=True)

                rz = ep.tile([P, 1], f32, tag="rz")
                nc.vector.reciprocal(rz[:], pt[:, OUT:OUT + 1])
                ob = obp.tile([P, OUT], f32, tag="ob")
                nc.vector.tensor_scalar(ob[:], pt[:, 0:OUT], rz[:], None,
                                        OP.mult)
                nc.sync.dma_start(outd[t * P:t * P + rows, :], ob[:rows, :])

    nc.compile()
    return nc


def kernel(x, edge_index, W, a1, a2):
    from concourse.bass_utils import run_bass_kernel_spmd

    x = np.asarray(x)
    edge_index = np.asarray(edge_index)
    W = np.asarray(W)
    a1 = np.asarray(a1)
    a2 = np.asarray(a2)

    N, IN = x.shape
    OUT = W.shape[0]
    pre = _preprocess(edge_index, N)
    NS, T = pre["NS"], pre["T"]
    NPAD = T * P

    nc = _build_program(N, IN, OUT, NS, T, pre["degpad"], pre["base"],
                        pre["SUMDEG"])

    Wt = W.T.astype(np.float32)
    wTn = np.ascontiguousarray(np.concatenate(
        [Wt, (Wt @ a1.astype(np.float32))[:, None],
         (Wt @ a2.astype(np.float32))[:, None]], axis=1)).astype(ml_dtypes.bfloat16)
    identn = np.eye(P, dtype=np.float32).astype(ml_dtypes.bfloat16)

    in_maps = []
    for c in range(C):
        orig = c * NS + pre["orders"][c]          # slot -> orig node id
        xs = np.zeros((NPAD, IN), dtype=np.float32)
        xs[:NS] = x[orig]
        in_maps.append({
            "xT": np.ascontiguousarray(xs.T).astype(ml_dtypes.bfloat16),
            "wT": wTn,
            "ident": identn,
            "offs": np.ascontiguousarray(pre["offs"][c]),
            "msk": np.ascontiguousarray(pre["msk"][c]),
        })

    res = run_bass_kernel_spmd(nc, in_maps, list(range(C)))

    out = np.empty((N, OUT), dtype=np.float32)
    for c in range(C):
        orig = c * NS + pre["orders"][c]
        out[orig] = res.results[c]["out"]
    return out
